# revision 19
# baseline (speedup 1.0000x reference)
"""EAGNN message-passing GNN on 8 Trainium2 NeuronCores (Bass/Tile).

Strategy (edge-parallel, dst-sharded):
  * Nodes are split into 8 contiguous ranges (one per core); edges are owned by
    the core that owns their dst node, sorted by dst and grouped into 128-node
    blocks so the segment-sum becomes per-block PSUM-accumulated matmuls with an
    on-chip one-hot selection matrix.  No cross-core reduction is needed.
  * The edge MLP's first layer is factored: concat([h_dst,h_src,e]) @ We1 ==
    (h@W_dst)[dst] + (h@W_src)[src] + e@W_e.  Per-node projections q = h@W_dst
    and r = h@W_src + be1 are computed node-parallel; r is AllGathered so every
    core can gather r[src] for its edge shard with indirect DMA.
  * The edge MLP's second layer is folded through the (linear) segment-sum:
    sum_e silu(m1_e) @ We2 == (sum_e silu(m1_e)) @ We2, applied per node block.
  * Everything on-chip is kept "feature-major" (H=128 on partitions) except the
    per-edge m1/silu tiles (edge-major) and the LayerNorm (node-major via a
    PE transpose).  bf16 storage / fp32 PSUM accumulation throughout.
"""

import math
import os
import sys

import numpy as np

for _p in ("/opt/trn_rl_repo", "/root/.axon_site/_ro/trn_rl_repo"):
    if os.path.isdir(_p) and _p not in sys.path:
        sys.path.insert(0, _p)

import ml_dtypes  # noqa: E402

import concourse.bass as bass  # noqa: E402
import concourse.bacc as bacc  # noqa: E402
import concourse.tile as tile  # noqa: E402
from concourse import mybir  # noqa: E402
from concourse.masks import make_identity  # noqa: E402

BF16 = mybir.dt.bfloat16
F32 = mybir.dt.float32
I32 = mybir.dt.int32
AF = mybir.ActivationFunctionType
ALU = mybir.AluOpType
NP_BF16 = ml_dtypes.bfloat16

H = 128
MACRO = 4  # 128-edge tiles per macro step (512 edges)


class Cfg:
    def __init__(self, C, NPC, NB, TPB, L, ENC_CHUNK=512, silu_native=True):
        self.silu_native = silu_native
        self.debug_taps = False
        self.C = C            # number of cores
        self.NPC = NPC        # real nodes per core (last core may have fewer)
        self.NB = NB          # 128-node blocks per core
        self.TPB = TPB        # 128-edge tiles per block (uniform, mult of MACRO)
        self.L = L
        self.NPAD = NB * 128
        assert TPB % MACRO == 0
        self.MPB = TPB // MACRO
        self.T_E = NB * TPB
        self.TE128 = self.T_E * 128
        self.ENC_CHUNK = ENC_CHUNK


# --------------------------------------------------------------------------
# program builder
# --------------------------------------------------------------------------

def build_program(cfg: Cfg):
    C, NB, TPB, L = cfg.C, cfg.NB, cfg.TPB, cfg.L
    NPAD, T_E, TE128, MPB = cfg.NPAD, cfg.T_E, cfg.TE128, cfg.MPB

    nc = bacc.Bacc("TRN2", target_bir_lowering=False)

    def inp(name, shape, dt):
        return nc.dram_tensor(name, shape, dt, kind="ExternalInput")[:]

    # per-core data
    ef_d = inp("ef", [4, TE128], BF16)
    srcidx_d = inp("srcidx", [128, T_E], I32)
    qidx_d = inp("qidx", [128, T_E], I32)
    dstloc_d = inp("dstloc", [128, T_E], F32)
    xfm_d = inp("xfm", [3, NPAD], BF16)
    screp_d = inp("screp", [128, NPAD], BF16)
    dbrow_d = inp("dbrow", [1, NPAD], BF16)
    # encoder / decoder weights
    wne1_d = inp("wne1", [3, 128], BF16)
    bne1_d = inp("bne1", [128, 1], BF16)
    wne2_d = inp("wne2", [128, 128], BF16)
    bne2_d = inp("bne2", [128, 1], BF16)
    wee1_d = inp("wee1", [4, 128], BF16)
    bee1_d = inp("bee1", [128, 1], BF16)
    wee2_d = inp("wee2", [128, 128], BF16)
    bee2_d = inp("bee2", [128, 1], BF16)
    wd1_d = inp("wd1", [128, 128], BF16)
    bd1_d = inp("bd1", [128, 1], BF16)
    wd2_d = inp("wd2", [128, 9], BF16)
    bd2_d = inp("bd2", [9, 1], BF16)
    # per-layer stacks, packed SBUF-style [partition, layer, free]
    wq_d = inp("wq", [128, L, 128], BF16)
    wr_d = inp("wr", [128, L, 128], BF16)
    we_d = inp("we", [128, L, 128], BF16)
    be1rep_d = inp("be1rep", [128, L, 128], BF16)
    we2_d = inp("we2", [128, L, 128], BF16)
    be2row_d = inp("be2row", [1, L * 128], BF16)
    wn1h_d = inp("wn1h", [128, L, 128], BF16)
    wn1a_d = inp("wn1a", [128, L, 128], BF16)
    bn1col_d = inp("bn1col", [128, L], BF16)
    wn2_d = inp("wn2", [128, L, 128], BF16)
    bn2col_d = inp("bn2col", [128, L], BF16)
    grep_d = inp("grep", [128, L, 128], BF16)
    brep_d = inp("brep", [128, L, 128], BF16)

    y_d = nc.dram_tensor("y", [9, NPAD], F32, kind="ExternalOutput")[:]
    taps = {}
    if cfg.debug_taps:
        for tn, shape in [("t_rstd", [128, 1]), ("t_zhat", [128, 128]),
                          ("t_hg", [128, 128]), ("t_h1b", [128, 128]),
                          ("t_ident", [128, 128]), ("t_zfm", [128, 128]),
                          ("t_znm", [128, 128]), ("t_mv", [128, 2]),
                          ("t_h0", [128, 128]), ("t_ea", [128, 512]),
                          ("t_q", [128, 128]), ("t_r", [128, 128]),
                          ("t_ag", [128, 128]), ("t_qg", [128, 512]),
                          ("t_rg", [128, 512]), ("t_m1", [128, 512]),
                          ("t_tem", [128, 512]), ("t_s4", [128, 512]),
                          ("t_agg", [128, 128]), ("t_aggf", [128, 128]),
                          ("t_h1", [128, 128])]:
            taps[tn] = nc.dram_tensor(tn, shape, F32, kind="ExternalOutput")[:]

    with tile.TileContext(nc) as tc:
        ctx = tc.ctx if hasattr(tc, "ctx") else None
        import contextlib

        stack = contextlib.ExitStack()
        with stack:
            dram = stack.enter_context(tc.tile_pool(name="dram", bufs=1, space="DRAM"))
            # internal DRAM buffers
            eattr_t = dram.tile([128, TE128], BF16, name="eattr")
            qtab_t = dram.tile([NPAD, 128], BF16, name="qtab")
            agin_t = dram.tile([NPAD, 128], BF16, name="agin")
            agout_ts = [
                dram.tile([C * NPAD, 128], BF16, name=f"agout{l}",
                          addr_space="Shared" if C > 4 else "Local")
                for l in range(L)
            ]

            singles = stack.enter_context(tc.tile_pool(name="singles", bufs=1))
            # persistent SBUF state
            h_sb = singles.tile([128, NPAD], BF16, name="h_sb")
            xfm_sb = singles.tile([3, NPAD], BF16, name="xfm_sb")
            screp_sb = singles.tile([128, NPAD], BF16, name="screp_sb")
            db_sb = singles.tile([1, NPAD], BF16, name="db_sb")
            srcidx_sb = singles.tile([128, T_E], I32, name="srcidx_sb")
            qidx_sb = singles.tile([128, T_E], I32, name="qidx_sb")
            dstloc_sb = singles.tile([128, T_E], F32, name="dstloc_sb")
            wne1_sb = singles.tile([3, 128], BF16, name="wne1_sb")
            bne1_sb = singles.tile([128, 1], BF16, name="bne1_sb")
            wne2_sb = singles.tile([128, 128], BF16, name="wne2_sb")
            bne2_sb = singles.tile([128, 1], BF16, name="bne2_sb")
            wee1_sb = singles.tile([4, 128], BF16, name="wee1_sb")
            bee1_sb = singles.tile([128, 1], BF16, name="bee1_sb")
            wee2_sb = singles.tile([128, 128], BF16, name="wee2_sb")
            bee2_sb = singles.tile([128, 1], BF16, name="bee2_sb")
            wd1_sb = singles.tile([128, 128], BF16, name="wd1_sb")
            bd1_sb = singles.tile([128, 1], BF16, name="bd1_sb")
            wd2_sb = singles.tile([128, 9], BF16, name="wd2_sb")
            bd2_sb = singles.tile([9, 1], BF16, name="bd2_sb")
            wq_sb = singles.tile([128, L, 128], BF16, name="wq_sb")
            wr_sb = singles.tile([128, L, 128], BF16, name="wr_sb")
            we_sb = singles.tile([128, L, 128], BF16, name="we_sb")
            be1rep_sb = singles.tile([128, L, 128], BF16, name="be1rep_sb")
            we2_sb = singles.tile([128, L, 128], BF16, name="we2_sb")
            be2_sb = singles.tile([1, L * 128], BF16, name="be2_sb")
            wn1h_sb = singles.tile([128, L, 128], BF16, name="wn1h_sb")
            wn1a_sb = singles.tile([128, L, 128], BF16, name="wn1a_sb")
            bn1_sb = singles.tile([128, L], BF16, name="bn1_sb")
            wn2_sb = singles.tile([128, L, 128], BF16, name="wn2_sb")
            bn2_sb = singles.tile([128, L], BF16, name="bn2_sb")
            grep_sb = singles.tile([128, L, 128], BF16, name="grep_sb")
            brep_sb = singles.tile([128, L, 128], BF16, name="brep_sb")
            iota_sb = singles.tile([128, 128], BF16, name="iota_sb")
            iota32_sb = singles.tile([128, 128], I32, name="iota32_sb")
            ident_sb = singles.tile([128, 128], BF16, name="ident_sb")
            eps_sb = singles.tile([128, 1], F32, name="eps_sb")
            nc.vector.memset(eps_sb[:], 1e-5)
            zero_sb = singles.tile([128, 1], F32, name="zero_sb")
            nc.vector.memset(zero_sb[:], 0.0)

            def emit_tap(name, src_ap, pool):
                if not cfg.debug_taps or name not in taps:
                    return
                shp = list(taps[name].shape)
                t = pool.tile(shp, F32, tag="tap_" + name)
                nc.vector.tensor_copy(t[:], src_ap)
                nc.sync.dma_start(out=taps[name], in_=t[:])

            def emit_silu(out_ap, in_ap, bias_ap, pool, tag, shape):
                # out = silu(in + bias); bias_ap is a per-partition column
                if cfg.silu_native:
                    nc.scalar.activation(out_ap, in_ap, AF.Silu, bias=bias_ap)
                else:
                    sg = pool.tile(shape, BF16, tag=tag)
                    sga = sg[:, : in_ap.shape[-1]] if len(shape) == 2 else sg[:]
                    nc.scalar.activation(sga, in_ap, AF.Sigmoid, bias=bias_ap)
                    nc.vector.scalar_tensor_tensor(
                        out_ap, in0=in_ap, scalar=bias_ap, in1=sga,
                        op0=ALU.add, op1=ALU.mult)

            for sb, d in [
                (xfm_sb, xfm_d), (screp_sb, screp_d), (db_sb, dbrow_d),
                (srcidx_sb, srcidx_d), (qidx_sb, qidx_d), (dstloc_sb, dstloc_d),
                (wne1_sb, wne1_d), (bne1_sb, bne1_d), (wne2_sb, wne2_d),
                (bne2_sb, bne2_d), (wee1_sb, wee1_d), (bee1_sb, bee1_d),
                (wee2_sb, wee2_d), (bee2_sb, bee2_d), (wd1_sb, wd1_d),
                (bd1_sb, bd1_d), (wd2_sb, wd2_d), (bd2_sb, bd2_d),
                (wq_sb, wq_d), (wr_sb, wr_d), (we_sb, we_d),
                (be1rep_sb, be1rep_d), (we2_sb, we2_d), (be2_sb, be2row_d),
                (wn1h_sb, wn1h_d), (wn1a_sb, wn1a_d), (bn1_sb, bn1col_d),
                (wn2_sb, wn2_d), (bn2_sb, bn2col_d), (grep_sb, grep_d),
                (brep_sb, brep_d),
            ]:
                nc.sync.dma_start(out=sb[:], in_=d)

            nc.gpsimd.iota(iota32_sb[:], pattern=[[1, 128]], base=0,
                           channel_multiplier=0)
            nc.vector.tensor_copy(iota_sb[:], iota32_sb[:])
            make_identity(nc, ident_sb[:])

            psum = stack.enter_context(
                tc.tile_pool(name="psum", bufs=3, space="PSUM"))
            psum_m1p = stack.enter_context(
                tc.tile_pool(name="psum_m1", bufs=2, space="PSUM"))
            psum_aggp = stack.enter_context(
                tc.tile_pool(name="psum_agg", bufs=2, space="PSUM"))
            work = stack.enter_context(tc.tile_pool(name="work", bufs=3))
            work2 = stack.enter_context(tc.tile_pool(name="work2", bufs=2))

            # ------------------------- node encoder -------------------------
            step = cfg.ENC_CHUNK
            for s in range(0, NPAD, step):
                w = min(step, NPAD - s)
                ps1 = psum.tile([128, step], F32, tag="ep")
                nc.tensor.matmul(out=ps1[:, :w], lhsT=wne1_sb[:],
                                 rhs=xfm_sb[:, s:s + w], start=True, stop=True)
                u = work.tile([128, step], BF16, tag="enc_u")
                emit_silu(u[:, :w], ps1[:, :w], bne1_sb[:, 0:1], work,
                          "enc_sg", [128, step])
                ps2 = psum.tile([128, step], F32, tag="ep")
                nc.tensor.matmul(out=ps2[:, :w], lhsT=wne2_sb[:],
                                 rhs=u[:, :w], start=True, stop=True)
                nc.scalar.activation(h_sb[:, s:s + w], ps2[:, :w], AF.Identity,
                                     bias=bne2_sb[:, 0:1])

            emit_tap("t_h0", h_sb[:, 0:128], work2)

            # ------------------------- edge encoder -------------------------
            for s in range(0, TE128, step):
                w = min(step, TE128 - s)
                efc = work.tile([4, step], BF16, tag="efc")
                nc.sync.dma_start(out=efc[:, :w], in_=ef_d[:, s:s + w])
                ps1 = psum.tile([128, step], F32, tag="ep")
                nc.tensor.matmul(out=ps1[:, :w], lhsT=wee1_sb[:],
                                 rhs=efc[:, :w], start=True, stop=True)
                u = work.tile([128, step], BF16, tag="enc_u")
                emit_silu(u[:, :w], ps1[:, :w], bee1_sb[:, 0:1], work,
                          "enc_sg", [128, step])
                ps2 = psum.tile([128, step], F32, tag="ep")
                nc.tensor.matmul(out=ps2[:, :w], lhsT=wee2_sb[:],
                                 rhs=u[:, :w], start=True, stop=True)
                ea = work.tile([128, step], BF16, tag="enc_ea")
                nc.scalar.activation(ea[:, :w], ps2[:, :w], AF.Identity,
                                     bias=bee2_sb[:, 0:1])
                nc.sync.dma_start(out=eattr_t[:, s:s + w], in_=ea[:, :w])

            # --------------------------- layers -----------------------------
            for l in range(L):
                if l == 1:
                    emit_tap("t_h1b", h_sb[:, 0:128], work2)
                # q / r projections (node-parallel over local blocks)
                for b in range(NB):
                    cols = slice(b * 128, (b + 1) * 128)
                    rows = slice(b * 128, (b + 1) * 128)
                    psq = psum.tile([128, 128], F32, tag="ep")
                    nc.tensor.matmul(out=psq[:], lhsT=h_sb[:, cols],
                                     rhs=wq_sb[:, l, :], start=True, stop=True)
                    qsb = work.tile([128, 128], BF16, tag="qsb")
                    nc.vector.tensor_copy(qsb[:], psq[:])
                    nc.sync.dma_start(out=qtab_t[rows, :], in_=qsb[:])
                    psr = psum.tile([128, 128], F32, tag="ep")
                    nc.tensor.matmul(out=psr[:], lhsT=h_sb[:, cols],
                                     rhs=wr_sb[:, l, :], start=True, stop=True)
                    rsb = work.tile([128, 128], BF16, tag="rsb")
                    nc.vector.tensor_tensor(rsb[:], psr[:], be1rep_sb[:, l, :],
                                            ALU.add)
                    nc.sync.dma_start(out=agin_t[rows, :], in_=rsb[:])
                    if l == 0 and b == 0:
                        emit_tap("t_q", qsb[:], work2)
                        emit_tap("t_r", rsb[:], work2)

                nc.gpsimd.collective_compute(
                    "AllGather", ALU.bypass,
                    replica_groups=[list(range(C))],
                    ins=[agin_t.opt()], outs=[agout_ts[l].opt()],
                )
                if l == 0 and cfg.debug_taps:
                    agt = work2.tile([128, 128], BF16, tag="agt")
                    nc.sync.dma_start(out=agt[:],
                                      in_=agout_ts[l][NPAD:NPAD + 128, :]
                                      if C > 1 else agout_ts[l][0:128, :])
                    emit_tap("t_ag", agt[:], work2)

                # edge phase
                for b in range(NB):
                    ps_agg = psum_aggp.tile([128, 128], F32, tag="agg")
                    for m in range(MPB):
                        mi = b * MPB + m
                        ec = mi * 512
                        tc4 = slice(mi * 4, mi * 4 + 4)
                        eat = work.tile([128, 512], BF16, tag="eat")
                        nc.sync.dma_start(out=eat[:], in_=eattr_t[:, ec:ec + 512])
                        qg = work.tile([128, 4, 128], BF16, tag="qg")
                        rg = work.tile([128, 4, 128], BF16, tag="rg")
                        for k in range(4):
                            nc.gpsimd.indirect_dma_start(
                                out=qg[:, k, :], out_offset=None,
                                in_=qtab_t[:],
                                in_offset=bass.IndirectOffsetOnAxis(
                                    ap=qidx_sb[:, mi * 4 + k:mi * 4 + k + 1],
                                    axis=0),
                            )
                            nc.gpsimd.indirect_dma_start(
                                out=rg[:, k, :], out_offset=None,
                                in_=agout_ts[l][:],
                                in_offset=bass.IndirectOffsetOnAxis(
                                    ap=srcidx_sb[:, mi * 4 + k:mi * 4 + k + 1],
                                    axis=0),
                            )
                        ps_m1 = psum_m1p.tile([128, 512], F32, tag="m1")
                        for k in range(4):
                            nc.tensor.matmul(
                                out=ps_m1[:, k * 128:(k + 1) * 128],
                                lhsT=eat[:, k * 128:(k + 1) * 128],
                                rhs=we_sb[:, l, :], start=True, stop=True)
                        s4 = work.tile([128, 4, 128], BF16, tag="s4")
                        for k in range(4):
                            nc.vector.tensor_scalar(
                                s4[:, k, :], iota_sb[:],
                                dstloc_sb[:, mi * 4 + k:mi * 4 + k + 1],
                                None, ALU.is_equal)
                        m1sb = work.tile([128, 512], BF16, tag="m1sb")
                        nc.vector.tensor_tensor(
                            m1sb[:],
                            qg[:].rearrange("p a b -> p (a b)"),
                            rg[:].rearrange("p a b -> p (a b)"), ALU.add)
                        nc.vector.tensor_tensor(
                            m1sb[:], m1sb[:], ps_m1[:], ALU.add)
                        tem = work.tile([128, 512], BF16, tag="tem")
                        emit_silu(tem[:], m1sb[:], zero_sb[:, 0:1], work,
                                  "sg", [128, 512])
                        if l == 0 and b == 0 and m == 0:
                            emit_tap("t_ea", eat[:], work2)
                            emit_tap("t_qg",
                                     qg[:].rearrange("p a b -> p (a b)"), work2)
                            emit_tap("t_rg",
                                     rg[:].rearrange("p a b -> p (a b)"), work2)
                            emit_tap("t_m1", m1sb[:], work2)
                            emit_tap("t_tem", tem[:], work2)
                            emit_tap("t_s4",
                                     s4[:].rearrange("p a b -> p (a b)"), work2)
                        for k in range(4):
                            nc.tensor.matmul(
                                out=ps_agg[:],
                                lhsT=tem[:, k * 128:(k + 1) * 128],
                                rhs=s4[:, k, :],
                                start=(m == 0 and k == 0),
                                stop=(m == MPB - 1 and k == 3))

                    # block epilogue: We2-fold, node MLP, residual, layernorm
                    cols = slice(b * 128, (b + 1) * 128)
                    if l == 0 and b == 0:
                        emit_tap("t_agg", ps_agg[:], work2)
                    aggs = work2.tile([128, 128], BF16, tag="aggs")
                    nc.vector.tensor_tensor(aggs[:], ps_agg[:],
                                            screp_sb[:, cols], ALU.mult)
                    ps_e = psum.tile([128, 128], F32, tag="ep")
                    nc.tensor.matmul(out=ps_e[:], lhsT=we2_sb[:, l, :],
                                     rhs=aggs[:], start=True, stop=False)
                    nc.tensor.matmul(out=ps_e[:],
                                     lhsT=be2_sb[:, l * 128:(l + 1) * 128],
                                     rhs=db_sb[:, cols],
                                     start=False, stop=True)
                    aggf = work2.tile([128, 128], BF16, tag="aggf")
                    nc.vector.tensor_copy(aggf[:], ps_e[:])
                    if l == 0 and b == 0:
                        emit_tap("t_aggf", aggf[:], work2)
                    ps_u = psum.tile([128, 128], F32, tag="ep")
                    nc.tensor.matmul(out=ps_u[:], lhsT=wn1h_sb[:, l, :],
                                     rhs=h_sb[:, cols], start=True, stop=False)
                    nc.tensor.matmul(out=ps_u[:], lhsT=wn1a_sb[:, l, :],
                                     rhs=aggf[:], start=False, stop=True)
                    u1 = work2.tile([128, 128], BF16, tag="u1")
                    emit_silu(u1[:], ps_u[:], bn1_sb[:, l:l + 1], work2,
                              "ep_sg", [128, 128])
                    ps_d = psum.tile([128, 128], F32, tag="ep")
                    nc.tensor.matmul(out=ps_d[:], lhsT=wn2_sb[:, l, :],
                                     rhs=u1[:], start=True, stop=True)
                    upd = work2.tile([128, 128], BF16, tag="upd")
                    nc.scalar.activation(upd[:], ps_d[:], AF.Identity,
                                         bias=bn2_sb[:, l:l + 1])
                    zfm = work2.tile([128, 128], BF16, tag="zfm")
                    nc.vector.tensor_tensor(zfm[:], upd[:], h_sb[:, cols],
                                            ALU.add)
                    ps_t = psum.tile([128, 128], BF16, tag="ep")
                    nc.tensor.transpose(ps_t[:], zfm[:], ident_sb[:])
                    znm = work2.tile([128, 128], F32, tag="znm")
                    nc.vector.tensor_copy(znm[:], ps_t[:])
                    st6 = work2.tile([128, 6], F32, tag="st6")
                    nc.vector.bn_stats(st6[:], znm[:])
                    mv = work2.tile([128, 2], F32, tag="mv")
                    nc.vector.bn_aggr(mv[:], st6[:])
                    if l == 0 and b == 0:
                        emit_tap("t_mv", mv[:], work2)
                    sd = work2.tile([128, 1], F32, tag="sd")
                    nc.scalar.activation(sd[:], mv[:, 1:2], AF.Sqrt,
                                         bias=eps_sb[:, 0:1])
                    rstd = work2.tile([128, 1], F32, tag="rstd")
                    nc.vector.reciprocal(rstd[:], sd[:])
                    zhat = work2.tile([128, 128], F32, tag="zhat")
                    nc.vector.tensor_scalar(zhat[:], znm[:],
                                            scalar1=mv[:, 0:1], scalar2=rstd[:],
                                            op0=ALU.subtract, op1=ALU.mult)
                    hg = work2.tile([128, 128], BF16, tag="hg")
                    nc.vector.tensor_tensor(hg[:], zhat[:], grep_sb[:, l, :],
                                            ALU.mult)
                    if l == 0 and b == 0:
                        emit_tap("t_rstd", rstd[:], work2)
                        emit_tap("t_zhat", zhat[:], work2)
                        emit_tap("t_hg", hg[:], work2)
                    hnm = work2.tile([128, 128], BF16, tag="hnm")
                    nc.vector.tensor_tensor(hnm[:], hg[:], brep_sb[:, l, :],
                                            ALU.add)
                    ps_t2 = psum.tile([128, 128], BF16, tag="ep")
                    nc.tensor.transpose(ps_t2[:], hnm[:], ident_sb[:])
                    nc.vector.tensor_copy(h_sb[:, cols], ps_t2[:])

            # --------------------------- decoder ----------------------------
            for b in range(NB):
                cols = slice(b * 128, (b + 1) * 128)
                ps1 = psum.tile([128, 128], F32, tag="ep")
                nc.tensor.matmul(out=ps1[:], lhsT=wd1_sb[:], rhs=h_sb[:, cols],
                                 start=True, stop=True)
                u2 = work2.tile([128, 128], BF16, tag="u2")
                emit_silu(u2[:], ps1[:], bd1_sb[:, 0:1], work2,
                          "ep_sg", [128, 128])
                ps2 = psum.tile([9, 128], F32, tag="ep")
                nc.tensor.matmul(out=ps2[:], lhsT=wd2_sb[:], rhs=u2[:],
                                 start=True, stop=True)
                ysb = work2.tile([9, 128], F32, tag="ysb")
                nc.scalar.activation(ysb[:], ps2[:], AF.Identity,
                                     bias=bd2_sb[:, 0:1])
                nc.sync.dma_start(out=y_d[:, cols], in_=ysb[:])

    nc.compile()
    return nc


# --------------------------------------------------------------------------
# host-side preprocessing
# --------------------------------------------------------------------------

def _bf(a):
    return np.ascontiguousarray(np.asarray(a, np.float32).astype(NP_BF16))


def host_prep(cfg: Cfg, inputs: dict):
    """Build the per-core in_maps (list of dicts) for run_bass_kernel_spmd."""
    C, NPC, NB, TPB, L = cfg.C, cfg.NPC, cfg.NB, cfg.TPB, cfg.L
    NPAD, T_E, TE128 = cfg.NPAD, cfg.T_E, cfg.TE128

    x = np.asarray(inputs["x"], np.float32)
    coords = np.asarray(inputs["coords"], np.float32)
    ei = np.asarray(inputs["edge_index"])
    N = x.shape[0]
    E = ei.shape[1]
    src = ei[0].astype(np.int64)
    dst = ei[1].astype(np.int64)

    deg = np.bincount(dst, minlength=N).astype(np.float32)
    sc = 1.0 / np.maximum(deg, 1.0)
    db = (deg > 0).astype(np.float32)
    rel = coords[dst] - coords[src]
    ef = np.concatenate([rel, np.linalg.norm(rel, axis=1, keepdims=True)], 1)

    core = dst // NPC
    dloc = dst - core * NPC
    blk = dloc // 128
    cb = (core * NB + blk).astype(np.int64)
    order = np.argsort(cb * (NPC + 1) + (dloc % 128) * 0 + dloc, kind="stable")
    # stable sort by (core, dloc):
    order = np.lexsort((dloc, core))
    counts = np.bincount(cb, minlength=C * NB)
    assert counts.max() <= TPB * 128, (
        f"TPB too small: need {math.ceil(counts.max() / 128)} tiles/block")

    sorted_cb = cb[order]
    starts = np.zeros(C * NB + 1, np.int64)
    starts[1:] = np.cumsum(counts)
    pos = np.arange(E, dtype=np.int64) - starts[sorted_cb]
    slot = (sorted_cb % NB) * TPB * 128 + pos  # slot within the core's arrays
    ecore = sorted_cb // NB

    # shared weight tensors
    def stackT(a):  # (L, 128, 128) -> (128, L, 128)
        return np.ascontiguousarray(np.transpose(np.asarray(a, np.float32),
                                                 (1, 0, 2))).astype(NP_BF16)

    We1 = np.asarray(inputs["We1"], np.float32)
    shared = {
        "wne1": _bf(inputs["W_ne1"]),
        "bne1": _bf(np.asarray(inputs["b_ne1"]).reshape(128, 1)),
        "wne2": _bf(inputs["W_ne2"]),
        "bne2": _bf(np.asarray(inputs["b_ne2"]).reshape(128, 1)),
        "wee1": _bf(inputs["W_ee1"]),
        "bee1": _bf(np.asarray(inputs["b_ee1"]).reshape(128, 1)),
        "wee2": _bf(inputs["W_ee2"]),
        "bee2": _bf(np.asarray(inputs["b_ee2"]).reshape(128, 1)),
        "wd1": _bf(inputs["Wd1"]),
        "bd1": _bf(np.asarray(inputs["bd1"]).reshape(128, 1)),
        "wd2": _bf(inputs["Wd2"]),
        "bd2": _bf(np.asarray(inputs["bd2"]).reshape(9, 1)),
        "wq": stackT(We1[:, 0:128, :]),
        "wr": stackT(We1[:, 128:256, :]),
        "we": stackT(We1[:, 256:384, :]),
        "be1rep": _bf(np.broadcast_to(
            np.asarray(inputs["be1"], np.float32)[None, :, :],
            (128, L, 128))),
        "we2": stackT(inputs["We2"]),
        "be2row": _bf(np.asarray(inputs["be2"], np.float32).reshape(1, L * 128)),
        "wn1h": stackT(np.asarray(inputs["Wn1"], np.float32)[:, 0:128, :]),
        "wn1a": stackT(np.asarray(inputs["Wn1"], np.float32)[:, 128:256, :]),
        "bn1col": _bf(np.asarray(inputs["bn1"], np.float32).T),
        "wn2": stackT(inputs["Wn2"]),
        "bn2col": _bf(np.asarray(inputs["bn2"], np.float32).T),
        "grep": _bf(np.broadcast_to(
            np.asarray(inputs["ln_g"], np.float32)[None, :, :], (128, L, 128))),
        "brep": _bf(np.broadcast_to(
            np.asarray(inputs["ln_b"], np.float32)[None, :, :], (128, L, 128))),
    }

    in_maps = []
    for c in range(C):
        msk = ecore == c
        sl = slot[msk]
        eids = order[msk]
        ef_pad = np.zeros((TE128, 4), np.float32)
        ef_pad[sl] = ef[eids]
        s_g = src[eids]
        srcslot = np.zeros(TE128, np.int32)
        srcslot[sl] = ((s_g // NPC) * NPAD + s_g % NPC).astype(np.int32)
        qidx = np.zeros(TE128, np.int32)
        qidx[sl] = (dst[eids] - c * NPC).astype(np.int32)
        dstloc = np.full(TE128, -1.0, np.float32)
        dstloc[sl] = ((dst[eids] - c * NPC) % 128).astype(np.float32)

        lo = c * NPC
        hi = min(N, (c + 1) * NPC)
        nvalid = hi - lo
        xfm = np.zeros((3, NPAD), np.float32)
        xfm[:, :nvalid] = x[lo:hi].T
        sc_l = np.zeros(NPAD, np.float32)
        sc_l[:nvalid] = sc[lo:hi]
        db_l = np.zeros(NPAD, np.float32)
        db_l[:nvalid] = db[lo:hi]

        m = dict(shared)
        m["ef"] = np.ascontiguousarray(ef_pad.T).astype(NP_BF16)
        m["srcidx"] = np.ascontiguousarray(srcslot.reshape(T_E, 128).T)
        m["qidx"] = np.ascontiguousarray(qidx.reshape(T_E, 128).T)
        m["dstloc"] = np.ascontiguousarray(dstloc.reshape(T_E, 128).T)
        m["xfm"] = xfm.astype(NP_BF16)
        m["screp"] = np.ascontiguousarray(
            np.broadcast_to(sc_l[None, :], (128, NPAD))).astype(NP_BF16)
        m["dbrow"] = db_l.reshape(1, NPAD).astype(NP_BF16)
        in_maps.append(m)
    return in_maps


def make_cfg(N, E, ei, C=8, L=6):
    NPC = math.ceil(N / C)
    NB = math.ceil(NPC / 128)
    dst = np.asarray(ei)[1].astype(np.int64)
    core = dst // NPC
    blk = (dst - core * NPC) // 128
    counts = np.bincount(core * NB + blk, minlength=C * NB)
    TPB = math.ceil(math.ceil(counts.max() / 128) / MACRO) * MACRO
    return Cfg(C=C, NPC=NPC, NB=NB, TPB=TPB, L=L)


_PROG_CACHE = {}


def kernel(**inputs) -> np.ndarray:
    from concourse.bass_utils import run_bass_kernel_spmd

    x = np.asarray(inputs["x"])
    ei = np.asarray(inputs["edge_index"])
    N, E = x.shape[0], ei.shape[1]
    L = np.asarray(inputs["We1"]).shape[0]
    cfg = make_cfg(N, E, ei, C=8, L=L)

    key = (cfg.C, cfg.NPC, cfg.NB, cfg.TPB, cfg.L)
    if key not in _PROG_CACHE:
        _PROG_CACHE[key] = build_program(cfg)
    nc = _PROG_CACHE[key]

    in_maps = host_prep(cfg, inputs)
    res = run_bass_kernel_spmd(nc, in_maps, list(range(cfg.C)))
    outs = []
    for c in range(cfg.C):
        lo = c * cfg.NPC
        hi = min(N, (c + 1) * cfg.NPC)
        y = np.asarray(res.results[c]["y"], np.float32)  # (9, NPAD)
        outs.append(y[:, : hi - lo].T)
    return np.concatenate(outs, axis=0).astype(np.float32)
